# revision 6
# baseline (speedup 1.0000x reference)
"""Trainium2 Bass kernel for nn_BiLSTMLag1 (4-layer BiLSTM + FC head).

Strategy (8 NeuronCores, SPMD):
  - Shard: 4 time-chunks (256 steps) x 2 batch-halves (512 samples).
  - Time-sharding is exact-to-bf16 via truncated warmup (W steps): LSTM state
    influence decays fast, so a chain started W steps early from zero state
    matches the full scan below the bf16 noise floor. Out-of-range warmup
    steps are masked by a per-timestep mask row (bias gate) + zero inputs,
    which keeps the state exactly zero, matching the reference init.
  - Layer cascade: layer l's valid output region extends (4-l)*W beyond the
    core's chunk so the next layer's warmup reads locally-computed data.
  - Inverted layout: batch (128) on partitions, gates on the free dim.
    Per step, per pair of chains (fwd+bwd), all 4 batch-tiles share one
    activation instruction. Gate matmul: lhsT = stacked input rows (data
    stationary), rhs = block-diagonal weight matrix giving both chains'
    gates of one batch tile in a single matmul; the recurrent part
    accumulates into the same PSUM.
  - The K=4 chain pairs of each layer are INTERLEAVED step-by-step
    (for s: for p:) so the per-step serial latency of one chain hides
    behind the other chains' work; the wall-clock approaches the busiest
    engine's total work instead of the summed chain latency.
  - DMA traffic is blocked: inputs are fetched and outputs flushed in
    8-step blocks (one DMA per chain per block instead of per step).
    Backward chains use negative-stride APs so descending-time blocks
    read/write with single DMAs; no mirrored copies are stored at all.
  - Gates use sigmoid only (tanh(g) = 2*sigmoid(2g)-1 with g-rows prescaled
    by 2 host-side); per-step h returns to matmul orientation via a PE
    transpose + one DVE copy into an 8-step rolling store that both feeds
    the recurrence and serves as the block flush source.
  - reps (for timing) is a hardware For_i loop: program size is constant
    in reps, so wall(reps=R) - wall(reps=1) isolates pure execution time
    (NEFF load/translation cost is identical for both programs).
"""

import numpy as np
import ml_dtypes

import concourse.bass as bass
import concourse.mybir as mybir
from concourse import bacc
from concourse.tile import TileContext
from concourse.masks import make_identity

BF16 = ml_dtypes.bfloat16
FP32 = mybir.dt.float32
BF = mybir.dt.bfloat16
AF = mybir.ActivationFunctionType
ALU = mybir.AluOpType

# layer dims: (din, H)
LAYERS = [(16, 20), (40, 20), (40, 10), (20, 10)]


class Cfg:
    def __init__(self, T=1024, B=1024, W=8, K=4, n_cores=8, reps=1,
                 use_for_i=True, interleave=True):
        self.T, self.B, self.W, self.K = T, B, W, K
        self.use_for_i = use_for_i
        self.interleave = interleave
        self.n_cores = n_cores
        self.reps = reps
        self.BS = W                  # DMA block size = warmup length
        self.n_tc = 4                # time chunks
        self.n_bh = n_cores // self.n_tc  # batch halves
        self.chunk = T // self.n_tc
        self.CB = B // self.n_bh     # batch per core
        self.NBT = self.CB // 128    # 128-row batch tiles per core
        self.TS = self.chunk + 8 * W  # local time-span of all buffers

    def valid(self, l):  # local [v0, v1) of layer l's output region
        return (l * self.W, self.TS - l * self.W)


def _gate_perm(H):
    # torch gate order i,f,g,o -> ours i,f,o,g
    return np.concatenate([np.arange(0, H), np.arange(H, 2 * H),
                           np.arange(3 * H, 4 * H), np.arange(2 * H, 3 * H)])


def _stg_rows(l):
    # rows per chain in the staging tile
    din, H = LAYERS[l]
    return (2 * 8 + 2) if l == 0 else (din + 1)


def _prep_weights(inputs):
    """Per layer: wx[l] [2*rpc, 2G] (x+bias block-diag) and wh[l] [2H, 2G]
    (recurrent block-diag). Gate cols reordered i,f,o,g, g-cols prescaled
    x2, bias on the mask rows. Layer 4 packs fwd weights in both halves
    (fwd-only pairs). Also w4b [din+1, G] for the single backward step of
    layer 4 (h=0 there, so no recurrent part).

    stg row layout per chain:
      l == 0: [x(t-1) (8); mask(t-1) (dead); x(t) (8); mask(t) -> bias]
      l >= 1: [h_f (H_in); h_b (H_in); mask -> bias]
    """
    wxs, whs = [], []
    for l, (din, H) in enumerate(LAYERS):
        G = 4 * H
        rpc = _stg_rows(l)
        perm = _gate_perm(H)
        mx = np.zeros((2 * rpc, 2 * G), np.float32)
        mh = np.zeros((2 * H, 2 * G), np.float32)
        li = l + 1
        for half in range(2):
            dr = "f" if (half == 0 or l == 3) else "b"
            wi = inputs[f"w{li}{dr}_ih"].astype(np.float32)[perm].T.copy()  # [din, G]
            wh = inputs[f"w{li}{dr}_hh"].astype(np.float32)[perm].T.copy()  # [H, G]
            b = (inputs[f"b{li}{dr}_ih"] + inputs[f"b{li}{dr}_hh"]).astype(np.float32)[perm].copy()
            wi[:, 3 * H:] *= 2.0; wh[:, 3 * H:] *= 2.0; b[3 * H:] *= 2.0
            c0 = half * G
            r0 = half * rpc
            if l == 0:
                mx[r0 + 0:r0 + 8, c0:c0 + G] = wi[8:16]   # lag features
                mx[r0 + 9:r0 + 17, c0:c0 + G] = wi[0:8]   # current features
                mx[r0 + 17, c0:c0 + G] = b                # mask(t) -> bias
            else:
                mx[r0:r0 + din, c0:c0 + G] = wi
                mx[r0 + din, c0:c0 + G] = b
            mh[half * H:(half + 1) * H, c0:c0 + G] = wh
        wxs.append(mx.astype(BF16))
        whs.append(mh.astype(BF16))
    # layer-4 backward single step: rows [h_f(10); h_b(10); mask->bias]
    din, H = LAYERS[3]
    G = 4 * H
    perm = _gate_perm(H)
    wi = inputs["w4b_ih"].astype(np.float32)[perm].T.copy()
    b = (inputs["b4b_ih"] + inputs["b4b_hh"]).astype(np.float32)[perm].copy()
    wi[:, 3 * H:] *= 2.0; b[3 * H:] *= 2.0
    w4b = np.zeros((din + 1, G), np.float32)
    w4b[0:din] = wi
    w4b[din] = b
    return wxs, whs, w4b.astype(BF16)


def _prep_xin(x, cfg, core):
    """Per-core input tensor [TS+1, 9, CB] bf16:
    slot i holds [x(t); mask(t)] for global t = c0 - 4W + i - 1
    (one slot of look-back so lag reads use slot i-1). Vectorized."""
    tc_, bh = core // cfg.n_bh, core % cfg.n_bh
    c0 = tc_ * cfg.chunk
    b0 = bh * cfg.CB
    base = c0 - 4 * cfg.W - 1          # global t of slot 0
    xin = np.zeros((cfg.TS + 1, 9, cfg.CB), BF16)
    t_lo = max(0, base)
    t_hi = min(cfg.T, base + cfg.TS + 1)
    i_lo, i_hi = t_lo - base, t_hi - base
    blk = np.transpose(x[b0:b0 + cfg.CB, t_lo:t_hi, :], (1, 2, 0))  # [t, 8, CB]
    xin[i_lo:i_hi, 0:8] = blk.astype(BF16)
    xin[i_lo:i_hi, 8] = 1.0
    return xin


# ------------------------- program builder -------------------------

def build_program(cfg):
    nc = bacc.Bacc(None, target_bir_lowering=False)
    NBT, TS, W, K, BS = cfg.NBT, cfg.TS, cfg.W, cfg.K, cfg.BS
    CB = cfg.CB

    xin = nc.declare_dram_parameter("xin", [TS + 1, 9, CB], BF, isOutput=False)
    wxd = [nc.declare_dram_parameter(f"wx{l}", [2 * _stg_rows(l), 8 * LAYERS[l][1]],
                                     BF, isOutput=False) for l in range(4)]
    whd = [nc.declare_dram_parameter(f"wh{l}", [2 * LAYERS[l][1], 8 * LAYERS[l][1]],
                                     BF, isOutput=False) for l in range(4)]
    w4b = nc.declare_dram_parameter("w4b", [LAYERS[3][0] + 1, 4 * LAYERS[3][1]],
                                    BF, isOutput=False)
    hf4out = nc.declare_dram_parameter("hf4out", [CB, 10], BF, isOutput=True)
    hb4out = nc.declare_dram_parameter("hb4out", [CB, 10], BF, isOutput=True)
    # lo[l][t] rows: [h_f(t) (H); h_b(t) (H); mask(t)]
    lo = [nc.dram_tensor(f"lo{l}", [TS, 2 * LAYERS[l][1] + 1, CB], BF)
          for l in range(3)]

    # per-layer spans
    subs, Ss = [], []
    for l in range(3):
        v0, v1 = cfg.valid(l + 1)
        span = v1 - v0
        assert span % K == 0
        subs.append(span // K)
        Ss.append(span // K + W)
    v0_4, v1_4 = cfg.valid(4)
    span4 = v1_4 - v0_4
    assert span4 % (2 * K) == 0
    sub4 = span4 // (2 * K)
    S4 = sub4 + W

    with TileContext(nc) as tc:
        with (
            tc.tile_pool(name="const", bufs=1) as constp,
            tc.tile_pool(name="stg", bufs=2 * cfg.K + 1) as stgp,
            tc.tile_pool(name="hst", bufs=2 * cfg.K + 1) as hstp,
            tc.tile_pool(name="sig", bufs=8) as sigp,
            tc.tile_pool(name="gc", bufs=cfg.K + 1) as gcp,
            tc.tile_pool(name="pp", bufs=8) as ppp,
            tc.tile_pool(name="tch", bufs=10) as tcp,
            tc.tile_pool(name="psg", bufs=3, space="PSUM") as psgp,
            tc.tile_pool(name="pst", bufs=2, space="PSUM") as pstp,
        ):
            ident = constp.tile([128, 128], BF, tag="ident")
            make_identity(nc, ident)
            wxt, wht = [], []
            for l in range(4):
                H_ = LAYERS[l][1]
                t_ = constp.tile([2 * _stg_rows(l), 8 * H_], BF, tag=f"wx{l}")
                nc.sync.dma_start(t_[:, :], wxd[l][:, :])
                wxt.append(t_)
                t_ = constp.tile([2 * H_, 8 * H_], BF, tag=f"wh{l}")
                nc.sync.dma_start(t_[:, :], whd[l][:, :])
                wht.append(t_)
            w4bt = constp.tile([LAYERS[3][0] + 1, 4 * LAYERS[3][1]], BF, tag="w4b")
            nc.sync.dma_start(w4bt[:, :], w4b[:, :])
            # mask prepass (rep-invariant): copy the mask row into each
            # layer-out buffer
            for l in range(3):
                H_ = LAYERS[l][1]
                nc.sync.dma_start(lo[l][:, 2 * H_:2 * H_ + 1, :],
                                  xin[1:TS + 1, 8:9, :])

            def t_starts(l, p, s):
                """(chainA time at step s, chainB time at step s)."""
                if l < 3:
                    a0 = (l + 1) * W + p * subs[l]
                    return a0 - W + s, a0 + subs[l] + W - 1 - s
                a0 = v0_4 + 2 * p * sub4
                b0 = v0_4 + (2 * p + 1) * sub4
                return a0 - W + s, b0 - W + s

            def fetch(l, p, blk, S):
                """Fetch one 8-step input block for pair p into a stg tile.
                Rows: chainA 0:rpc, chainB rpc:2rpc; slice j = step blk*BS+j."""
                rpc = _stg_rows(l)
                blen = min(BS, S - blk * BS)
                sA = blk * BS
                ta, tb = t_starts(l, p, sA)
                stg = stgp.tile([2 * rpc, BS, NBT, 128], BF, tag="stg", name=f"stg{l}_{p}_{blk}")
                if l == 0:
                    # xin slot for time t is t+1; lag = slot t
                    nc.gpsimd.dma_start(
                        stg[0:9, 0:blen, :, :],
                        xin[ta:ta + blen].rearrange("t r b -> r t b"))
                    nc.gpsimd.dma_start(
                        stg[9:18, 0:blen, :, :],
                        xin[ta + 1:ta + 1 + blen].rearrange("t r b -> r t b"))
                    bstop = tb - blen
                    bsl = slice(tb, None, -1) if bstop < 0 else slice(tb, bstop, -1)
                    nc.gpsimd.dma_start(
                        stg[18:27, 0:blen, :, :],
                        xin[bsl].rearrange("t r b -> r t b"))
                    nc.gpsimd.dma_start(
                        stg[27:36, 0:blen, :, :],
                        xin[tb + 1:tb + 1 - blen:-1].rearrange("t r b -> r t b"))
                else:
                    src = lo[l - 1]
                    nc.gpsimd.dma_start(
                        stg[0:rpc, 0:blen, :, :],
                        src[ta:ta + blen].rearrange("t r b -> r t b"))
                    if l < 3:
                        nc.gpsimd.dma_start(
                            stg[rpc:2 * rpc, 0:blen, :, :],
                            src[tb:tb - blen:-1].rearrange("t r b -> r t b"))
                    else:
                        nc.gpsimd.dma_start(
                            stg[rpc:2 * rpc, 0:blen, :, :],
                            src[tb:tb + blen].rearrange("t r b -> r t b"))
                return stg

            def flush(l, p, blk, hst, S):
                """Write one 8-step block of h outputs to lo[l] (l < 3)."""
                H = LAYERS[l][1]
                blen = min(BS, S - blk * BS)
                sA = blk * BS
                ta, tb = t_starts(l, p, sA)
                nc.sync.dma_start(
                    lo[l][ta:ta + blen, 0:H, :].rearrange("t r b -> r t b"),
                    hst[0:H, 0:blen, :, :])
                nc.sync.dma_start(
                    lo[l][tb:tb - blen:-1, H:2 * H, :].rearrange("t r b -> r t b"),
                    hst[H:2 * H, 0:blen, :, :])

            def step(l, p, s, S, stg, hst_c, hst_p, gc, grab_hf4):
                din, H = LAYERS[l]
                G = 4 * H
                j = s % BS
                # 256-col stride per batch-tile keeps each matmul output
                # inside one PSUM bank
                gps = psgp.tile([128, NBT, 256], FP32, tag="gps")
                for bt in range(NBT):
                    nc.tensor.matmul(gps[:, bt, 0:2 * G], stg[:, j, bt, :],
                                     wxt[l][:, :], start=True, stop=(s == 0))
                    if s > 0:
                        hsrc = hst_p if j == 0 else hst_c
                        nc.tensor.matmul(gps[:, bt, 0:2 * G],
                                         hsrc[:, (j - 1) % BS, bt, :],
                                         wht[l][:, :], start=False, stop=True)
                sig = sigp.tile([128, NBT, 2, G], BF, tag="sig")
                nc.scalar.activation(sig[:, :, :, :], gps[:, :, 0:2 * G], AF.Sigmoid)
                # gtil(s) = 2*sigma(2g)-1 overwrites the dead gtil(s-1)
                nc.vector.tensor_scalar(gc[:, :, :, 0:H],
                                        sig[:, :, :, 3 * H:4 * H],
                                        2.0, -1.0, ALU.mult, ALU.add)
                if s == 0:
                    prod = ppp.tile([128, NBT, 2, H], BF, tag="pp0")
                    nc.vector.tensor_tensor(prod[:, :, :, :],
                                            sig[:, :, :, 0:H],
                                            gc[:, :, :, 0:H], ALU.mult)
                    nc.vector.tensor_copy(gc[:, :, :, H:2 * H],
                                          prod[:, :, :, :])
                else:
                    prod = ppp.tile([128, NBT, 2, 2 * H], BF, tag="pp")
                    nc.vector.tensor_tensor(prod[:, :, :, :],
                                            sig[:, :, :, 0:2 * H],
                                            gc[:, :, :, :], ALU.mult)
                    nc.vector.tensor_tensor(gc[:, :, :, H:2 * H],
                                            prod[:, :, :, 0:H],
                                            prod[:, :, :, H:2 * H], ALU.add)
                tch = tcp.tile([128, NBT, 2, H], BF, tag="tch")
                nc.scalar.activation(tch[:, :, :, :], gc[:, :, :, H:2 * H],
                                     AF.Tanh)
                hs = tcp.tile([128, NBT, 2, H], BF, tag="hs")
                nc.vector.tensor_tensor(hs[:, :, :, :],
                                        sig[:, :, :, 2 * H:3 * H],
                                        tch[:, :, :, :], ALU.mult)
                if grab_hf4:
                    for bt in range(NBT):
                        nc.sync.dma_start(hf4out[bt * 128:(bt + 1) * 128, :],
                                          hs[:, bt, 1, :])
                tps = pstp.tile([2 * H, NBT, 128], BF, tag="tps")
                for bt in range(NBT):
                    nc.tensor.transpose(tps[:, bt, :], hs[:, bt, :, :],
                                        ident[:, :])
                nc.vector.tensor_copy(hst_c[:, j, :, :], tps[:, :, :])

            cfg_interleave = cfg.interleave

            def layer_phase(l):
                H = LAYERS[l][1]
                S = Ss[l] if l < 3 else S4
                nblk = (S + BS - 1) // BS
                stg_t, hst_c, hst_p, gcs = {}, {}, {}, {}
                for p in range(K):
                    stg_t[p] = fetch(l, p, 0, S)
                    gcs[p] = gcp.tile([128, NBT, 2, 2 * H], BF, tag="gc", name=f"gc{l}_{p}")
                    hst_c[p] = hstp.tile([2 * H, BS, NBT, 128], BF, tag="hst", name=f"hst{l}_{p}_0")
                    hst_p[p] = None
                for blk in range(nblk):
                    blen = min(BS, S - blk * BS)
                    stg_n = {}
                    for p in range(K):
                        if blk + 1 < nblk:
                            stg_n[p] = fetch(l, p, blk + 1, S)
                    if cfg_interleave:
                        for jj in range(blen):
                            s = blk * BS + jj
                            for p in range(K):
                                grab = (l == 3 and p == K - 1 and s == S - 1)
                                step(l, p, s, S, stg_t[p], hst_c[p], hst_p[p],
                                     gcs[p], grab)
                    else:
                        for p in range(K):
                            for jj in range(blen):
                                s = blk * BS + jj
                                grab = (l == 3 and p == K - 1 and s == S - 1)
                                step(l, p, s, S, stg_t[p], hst_c[p], hst_p[p],
                                     gcs[p], grab)
                    for p in range(K):
                        if l < 3 and blk >= 1:
                            flush(l, p, blk, hst_c[p], S)
                        if blk + 1 < nblk:
                            stg_t[p] = stg_n[p]
                            hst_p[p] = hst_c[p]
                            hst_c[p] = hstp.tile([2 * H, BS, NBT, 128], BF,
                                                 tag="hst", name=f"hst{l}_{p}_{blk + 1}")
                    # scheduler-only fence: bounds the reorder window so
                    # tile scheduling stays tractable
                    tc.no_sync_barrier()

            def run_phases():
                for l in range(4):
                    layer_phase(l)
                # ---- layer 4 backward: single step at the last timestep ----
                tlast = v1_4 - 1
                din, H = LAYERS[3]
                G = 4 * H
                stg1 = stgp.tile([din + 1, NBT, 128], BF, tag="stg1")
                nc.sync.dma_start(stg1[:, :, :], lo[2][tlast, :, :])
                gps = psgp.tile([128, NBT, 256], FP32, tag="gps")
                for bt in range(NBT):
                    nc.tensor.matmul(gps[:, bt, 0:G], stg1[:, bt, :],
                                     w4bt[:, :], start=True, stop=True)
                sig = sigp.tile([128, NBT, G], BF, tag="sig4b")
                nc.scalar.activation(sig[:, :, :], gps[:, :, 0:G], AF.Sigmoid)
                gt = tcp.tile([128, NBT, H], BF, tag="gt4b")
                nc.vector.tensor_scalar(gt[:, :, :], sig[:, :, 3 * H:4 * H],
                                        2.0, -1.0, ALU.mult, ALU.add)
                cc = tcp.tile([128, NBT, H], BF, tag="cc4b")
                nc.vector.tensor_tensor(cc[:, :, :], sig[:, :, 0:H],
                                        gt[:, :, :], ALU.mult)
                tch = tcp.tile([128, NBT, H], BF, tag="tch4b")
                nc.scalar.activation(tch[:, :, :], cc[:, :, :], AF.Tanh)
                hb1 = tcp.tile([128, NBT, H], BF, tag="hb4b")
                nc.vector.tensor_tensor(hb1[:, :, :], sig[:, :, 2 * H:3 * H],
                                        tch[:, :, :], ALU.mult)
                for bt in range(NBT):
                    nc.sync.dma_start(hb4out[bt * 128:(bt + 1) * 128, :],
                                      hb1[:, bt, :])

            # hardware reps loop: program size constant in reps, so
            # wall(reps=R) - wall(reps=1) isolates per-rep execution.
            if cfg.use_for_i:
                with tc.For_i(0, cfg.reps, 1):
                    run_phases()
            else:
                for _rep in range(cfg.reps):
                    run_phases()
    nc.compile()
    return nc


# ------------------------- entry point -------------------------

_CACHE = {}


def _get_program(cfg):
    key = (cfg.T, cfg.B, cfg.W, cfg.K, cfg.reps)
    if key not in _CACHE:
        _CACHE[key] = build_program(cfg)
    return _CACHE[key]


def kernel(_cfg=None, _trace=False, **inputs):
    from concourse.bass_utils import run_bass_kernel_spmd

    cfg = _cfg or Cfg()
    x = np.asarray(inputs["x"])
    wxs, whs, w4bm = _prep_weights(inputs)
    nc = _get_program(cfg)

    in_maps = []
    for core in range(cfg.n_cores):
        m = {"xin": _prep_xin(x, cfg, core), "w4b": w4bm}
        for l in range(4):
            m[f"wx{l}"] = wxs[l]
            m[f"wh{l}"] = whs[l]
        in_maps.append(m)

    import time
    t0 = time.perf_counter()
    res = run_bass_kernel_spmd(nc, in_maps, list(range(cfg.n_cores)),
                               trace=_trace)
    kernel.last_wall_s = time.perf_counter() - t0
    results = res.results
    kernel.last_exec_time_ns = res.exec_time_ns

    # gather: last time-chunk cores hold t = T-1
    h4 = np.zeros((cfg.B, 20), np.float32)
    for bh in range(cfg.n_bh):
        core = (cfg.n_tc - 1) * cfg.n_bh + bh
        b0 = bh * cfg.CB
        h4[b0:b0 + cfg.CB, 0:10] = results[core]["hf4out"].astype(np.float32)
        h4[b0:b0 + cfg.CB, 10:20] = results[core]["hb4out"].astype(np.float32)

    fc_w = np.asarray(inputs["fc_w"], np.float32)
    fc_b = np.asarray(inputs["fc_b"], np.float32)
    z = h4 @ fc_w.T + fc_b
    return (1.0 / (1.0 + np.exp(-z))).astype(np.float32)
